# revision 2
# baseline (speedup 1.0000x reference)
"""Trainium2 Bass kernel for the ContentAM module (dense transformer attention block).

Reference computation (per batch b):
    Q = Wq @ x[b] + bq            [D=128, N=4096]
    K = Wk @ x[b] + bk            [D=128, N=4096]
    V = Wv @ x1[b] + bv           [C=1024, N=4096]
    att = sigmoid(Q^T K)          [N, N]
    out = V @ att^T               [C, N]
    gate = sigmoid(MLP(mean_n(out)) + MLP(max_n(out)))   (CBAM channel attention)
    final = out * gate[:, None]

Sharding: 8 cores = 4 batches x 2 query-halves. Each core computes Q for its
2048-column half, the full K and V^T, attT = sigmoid(K^T Q) for its half, and
out_half via the m-contraction. The pipeline is transpose-free: we compute
attT = K^T Q (not Q^T K) and VT = x1^T Wv^T (not Wv x1), so every matmul's
operands are produced in the layout the next matmul consumes.

The SPMD program is identical on all cores; the query-half split is realized
by permuting the m-columns of x/x1 on the host (odd cores see their half
first), which is an internally consistent relabeling of the contracted m axis.

CBAM pooling spans both halves of a pair, exchanged with a single 8KB
AllGather over replica pairs; the MLP runs on-device in fp32, with the
mean scale folded into the final sigmoid (relu is positively homogeneous).

All matmul operands are fp16 (fp32 PSUM accumulation); measured end-to-end
rel_l2 vs the fp32 reference is ~1.3e-3.
"""

import sys

sys.path.insert(0, "/opt/trn_rl_repo")

import numpy as np

import concourse.bass as bass
import concourse.bacc as bacc
import concourse.mybir as mybir
import concourse.tile as tile
from concourse.bass_utils import run_bass_kernel_spmd

# Problem shapes (hardcoded per contract)
B = 4
C = 1024
D = 128
R = 64
N = 4096
NH = N // 2
P = 128
KC = C // P          # 8 contraction k-tiles over channels
NCORES = 8
CHUNK = 8            # m-tiles per out-accumulation chunk
NCHUNK = (N // P) // CHUNK   # 4

F16 = mybir.dt.float16
F32 = mybir.dt.float32

# toggles/results for the test harness
ENABLE_TRACE = False
LAST_EXEC_NS = None
LAST_TRACE_PATH = None

_CACHED_NC = None


def _build_program():
    nc = bacc.Bacc("TRN2", target_bir_lowering=False, debug=False, num_devices=NCORES)

    xb = nc.dram_tensor("xb", [C, N], F16, kind="ExternalInput").ap()
    x1b = nc.dram_tensor("x1b", [C, N], F16, kind="ExternalInput").ap()
    wqt = nc.dram_tensor("wqt", [C, D], F16, kind="ExternalInput").ap()
    wkt = nc.dram_tensor("wkt", [C, D], F16, kind="ExternalInput").ap()
    wvt = nc.dram_tensor("wvt", [C, C], F16, kind="ExternalInput").ap()
    bq = nc.dram_tensor("bq", [D, 1], F32, kind="ExternalInput").ap()
    bk = nc.dram_tensor("bk", [D, 1], F32, kind="ExternalInput").ap()
    bv = nc.dram_tensor("bv", [C], F32, kind="ExternalInput").ap()
    wca1t = nc.dram_tensor("wca1t", [C, R], F32, kind="ExternalInput").ap()
    wca2t = nc.dram_tensor("wca2t", [R, C], F32, kind="ExternalInput").ap()
    outd = nc.dram_tensor("out", [C, NH], F32, kind="ExternalOutput").ap()

    xb_r = xb.rearrange("(k p) n -> p k n", p=P)
    x1b_r = x1b.rearrange("(k p) n -> p k n", p=P)

    with tile.TileContext(nc) as tc:
        with (
            tc.tile_pool(name="const", bufs=1) as const,
            tc.tile_pool(name="xs", bufs=3) as xs_pool,
            tc.tile_pool(name="x1s", bufs=3) as x1s_pool,
            tc.tile_pool(name="qk", bufs=1) as qk_pool,
            tc.tile_pool(name="kch", bufs=2) as kch_pool,
            tc.tile_pool(name="attp", bufs=9) as att_pool,
            tc.tile_pool(name="vtp", bufs=9) as vt_pool,
            tc.tile_pool(name="outp", bufs=8) as out_pool,
            tc.tile_pool(name="small", bufs=2) as small,
            tc.tile_pool(name="ps_qk", bufs=2, space="PSUM") as ps_qk,
            tc.tile_pool(name="ps_vt", bufs=2, space="PSUM") as ps_vt,
            tc.tile_pool(name="ps_att", bufs=2, space="PSUM") as ps_att,
            tc.tile_pool(name="ps_out", bufs=2, space="PSUM") as ps_out,
            tc.tile_pool(name="dram", bufs=1, space="DRAM") as dram,
        ):
            # ---- constants ----
            wqt_t = const.tile([P, KC, D], F16, tag="wqt")
            wkt_t = const.tile([P, KC, D], F16, tag="wkt")
            wvt_t = const.tile([P, KC, C], F16, tag="wvt")
            wca1_t = const.tile([P, KC, R], F32, tag="wca1")
            wca2_t = const.tile([R, C], F32, tag="wca2")
            bq_t = const.tile([P, 1], F32, tag="bq")
            bk_t = const.tile([P, 1], F32, tag="bk")
            bv_bc = const.tile([P, C], F32, tag="bvbc")
            nc.sync.dma_start(wqt_t[:], wqt.rearrange("(k p) d -> p k d", p=P))
            nc.sync.dma_start(wkt_t[:], wkt.rearrange("(k p) d -> p k d", p=P))
            nc.sync.dma_start(wvt_t[:], wvt.rearrange("(k p) c -> p k c", p=P))
            nc.sync.dma_start(wca1_t[:], wca1t.rearrange("(k p) r -> p k r", p=P))
            nc.sync.dma_start(wca2_t[:], wca2t[:])
            nc.sync.dma_start(bq_t[:], bq[:])
            nc.sync.dma_start(bk_t[:], bk[:])
            nc.sync.dma_start(bv_bc[:], bv[None, :].to_broadcast((P, C)))

            q_sb = const.tile([P, NH], F16, tag="qsb")
            out_sb = [
                const.tile([P, NH], F32, tag=f"osb{ct}", name=f"osb{ct}")
                for ct in range(8)
            ]

            # ---- phase Q: our query half lives in columns [0, NH) ----
            for t in range(NH // 512):
                xq = xs_pool.tile([P, KC, 512], F16, tag="xs")
                nc.sync.dma_start(xq[:], xb_r[:, :, t * 512:(t + 1) * 512])
                q_ps = ps_qk.tile([P, 512], F32, tag="qk")
                for k in range(KC):
                    nc.tensor.matmul(
                        q_ps[:], wqt_t[:, k, :], xq[:, k, :],
                        start=(k == 0), stop=(k == KC - 1),
                    )
                nc.scalar.activation(
                    q_sb[:, t * 512:(t + 1) * 512], q_ps[:],
                    mybir.ActivationFunctionType.Identity, bias=bq_t[:],
                )

            # ---- main loop over m chunks ----
            for ch in range(NCHUNK):
                k_ch = kch_pool.tile([P, CHUNK * P], F16, tag="kch")
                chunk_vt = []
                chunk_att = []
                for g in range(2):  # two 512-col groups per chunk
                    cols = ch * 1024 + g * 512
                    xg = xs_pool.tile([P, KC, 512], F16, tag="xs")
                    nc.sync.dma_start(xg[:], xb_r[:, :, cols:cols + 512])
                    k_ps = ps_qk.tile([P, 512], F32, tag="qk")
                    for k in range(KC):
                        nc.tensor.matmul(
                            k_ps[:], wkt_t[:, k, :], xg[:, k, :],
                            start=(k == 0), stop=(k == KC - 1),
                        )
                    nc.scalar.activation(
                        k_ch[:, g * 512:(g + 1) * 512], k_ps[:],
                        mybir.ActivationFunctionType.Identity, bias=bk_t[:],
                    )
                    for half in range(2):  # two 256-col x1 loads per group
                        mcols = cols + half * 256
                        x1g = x1s_pool.tile([P, KC, 256], F16, tag="x1s")
                        nc.sync.dma_start(x1g[:], x1b_r[:, :, mcols:mcols + 256])
                        for j in range(2):  # m-tiles within the load
                            lm = g * 4 + half * 2 + j
                            vt_t = vt_pool.tile([P, C], F16, tag="vt")
                            for cg in range(2):
                                vt_ps = ps_vt.tile([P, 512], F32, tag="vt")
                                for k in range(KC):
                                    nc.tensor.matmul(
                                        vt_ps[:],
                                        x1g[:, k, j * P:(j + 1) * P],
                                        wvt_t[:, k, cg * 512:(cg + 1) * 512],
                                        start=(k == 0), stop=(k == KC - 1),
                                    )
                                nc.vector.tensor_add(
                                    vt_t[:, cg * 512:(cg + 1) * 512], vt_ps[:],
                                    bv_bc[:, cg * 512:(cg + 1) * 512],
                                )
                            att_t = att_pool.tile([P, NH], F16, tag="att")
                            for nj in range(NH // 512):
                                att_ps = ps_att.tile([P, 512], F32, tag="att")
                                nc.tensor.matmul(
                                    att_ps[:],
                                    k_ch[:, lm * P:(lm + 1) * P],
                                    q_sb[:, nj * 512:(nj + 1) * 512],
                                    start=True, stop=True,
                                )
                                nc.scalar.activation(
                                    att_t[:, nj * 512:(nj + 1) * 512], att_ps[:],
                                    mybir.ActivationFunctionType.Sigmoid,
                                )
                            chunk_vt.append(vt_t)
                            chunk_att.append(att_t)

                # out += VT_chunk^T-contraction over this chunk's 8 m-tiles
                for ct in range(8):
                    for nj in range(NH // 512):
                        out_ps = ps_out.tile([P, 512], F32, tag="out")
                        for j in range(CHUNK):
                            nc.tensor.matmul(
                                out_ps[:],
                                chunk_vt[j][:, ct * P:(ct + 1) * P],
                                chunk_att[j][:, nj * 512:(nj + 1) * 512],
                                start=(j == 0), stop=(j == CHUNK - 1),
                            )
                        dst = out_sb[ct][:, nj * 512:(nj + 1) * 512]
                        if ch == 0:
                            nc.vector.tensor_copy(dst, out_ps[:])
                        else:
                            nc.vector.tensor_add(dst, dst, out_ps[:])

            # ---- CBAM channel attention ----
            stats = small.tile([P, 8, 2], F32, tag="stats")
            for ct in range(8):
                nc.vector.reduce_sum(
                    stats[:, ct, 0:1], out_sb[ct][:], axis=mybir.AxisListType.X
                )
                nc.vector.reduce_max(
                    stats[:, ct, 1:2], out_sb[ct][:], axis=mybir.AxisListType.X
                )

            cc_in = dram.tile([P, 16], F32, tag="ccin")
            cc_out = dram.tile([2, P, 16], F32, tag="ccout")
            nc.sync.dma_start(cc_in[:], stats[:])
            nc.gpsimd.collective_compute(
                "AllGather",
                mybir.AluOpType.bypass,
                replica_groups=[[0, 1], [2, 3], [4, 5], [6, 7]],
                ins=[cc_in[:].opt()],
                outs=[cc_out[:].opt()],
            )
            g0 = small.tile([P, 8, 2], F32, tag="g0")
            g1 = small.tile([P, 8, 2], F32, tag="g1")
            nc.sync.dma_start(g0[:], cc_out[0])
            nc.sync.dma_start(g1[:], cc_out[1])
            comb = small.tile([P, 8, 2], F32, tag="comb")
            nc.vector.tensor_add(comb[:, :, 0:1], g0[:, :, 0:1], g1[:, :, 0:1])
            nc.vector.tensor_max(comb[:, :, 1:2], g0[:, :, 1:2], g1[:, :, 1:2])

            # layer 1: [R, 2] = Wca1 @ (sum_total, max_total); fp32 matmul
            u1_ps = ps_att.tile([R, 2], F32, tag="att")
            for k in range(KC):
                nc.tensor.matmul(
                    u1_ps[:], wca1_t[:, k, :], comb[:, k, :],
                    start=(k == 0), stop=(k == KC - 1),
                )
            u1_sb = small.tile([R, 2], F32, tag="u1")
            nc.scalar.activation(u1_sb[:], u1_ps[:], mybir.ActivationFunctionType.Relu)

            gate_sb = small.tile([P, 8], F32, tag="gate")
            for ct in range(8):
                g_ps = ps_out.tile([P, 2], F32, tag="out")
                nc.tensor.matmul(
                    g_ps[:], wca2_t[:, ct * P:(ct + 1) * P], u1_sb[:],
                    start=True, stop=True,
                )
                mx_col = small.tile([P, 1], F32, tag="mxcol")
                nc.vector.tensor_copy(mx_col[:], g_ps[:, 1:2])
                # gate = sigmoid(mlp(sum)/N_total + mlp(max)); relu is
                # positively homogeneous so the mean scale folds in here
                nc.scalar.activation(
                    gate_sb[:, ct:ct + 1], g_ps[:, 0:1],
                    mybir.ActivationFunctionType.Sigmoid,
                    scale=1.0 / N, bias=mx_col[:],
                )

            for ct in range(8):
                nc.vector.tensor_scalar_mul(
                    out_sb[ct][:], out_sb[ct][:], gate_sb[:, ct:ct + 1]
                )
                nc.sync.dma_start(outd[ct * P:(ct + 1) * P, :], out_sb[ct][:])

    nc.compile()
    return nc


def _get_program():
    global _CACHED_NC
    if _CACHED_NC is None:
        _CACHED_NC = _build_program()
    return _CACHED_NC


def kernel(x, x1, Wq, bq, Wk, bk, Wv, bv, Wca1, Wca2):
    global LAST_EXEC_NS, LAST_TRACE_PATH
    x = np.asarray(x, dtype=np.float32)
    x1 = np.asarray(x1, dtype=np.float32)
    Wq = np.asarray(Wq, dtype=np.float32)
    Wk = np.asarray(Wk, dtype=np.float32)
    Wv = np.asarray(Wv, dtype=np.float32)
    bq = np.asarray(bq, dtype=np.float32)
    bk = np.asarray(bk, dtype=np.float32)
    bv = np.asarray(bv, dtype=np.float32)
    Wca1 = np.asarray(Wca1, dtype=np.float32)
    Wca2 = np.asarray(Wca2, dtype=np.float32)

    wqt16 = np.ascontiguousarray(Wq.T).astype(np.float16)
    wkt16 = np.ascontiguousarray(Wk.T).astype(np.float16)
    wvt16 = np.ascontiguousarray(Wv.T).astype(np.float16)
    wca1t = np.ascontiguousarray(Wca1.T)
    wca2t = np.ascontiguousarray(Wca2.T)
    bq2 = bq.reshape(D, 1)
    bk2 = bk.reshape(D, 1)

    in_maps = []
    for core in range(NCORES):
        b, h = divmod(core, 2)
        xb = x[b].reshape(C, N).astype(np.float16)
        x1b = x1[b].reshape(C, N).astype(np.float16)
        if h == 1:
            # put this core's query half first; consistent m-relabeling
            xb = np.concatenate([xb[:, NH:], xb[:, :NH]], axis=1)
            x1b = np.concatenate([x1b[:, NH:], x1b[:, :NH]], axis=1)
        in_maps.append({
            "xb": np.ascontiguousarray(xb),
            "x1b": np.ascontiguousarray(x1b),
            "wqt": wqt16, "wkt": wkt16, "wvt": wvt16,
            "bq": bq2, "bk": bk2, "bv": bv,
            "wca1t": wca1t, "wca2t": wca2t,
        })

    nc = _get_program()
    res = run_bass_kernel_spmd(
        nc, in_maps, list(range(NCORES)), trace=ENABLE_TRACE
    )
    LAST_EXEC_NS = res.exec_time_ns
    LAST_TRACE_PATH = (
        res.instructions_and_trace[1] if res.instructions_and_trace else None
    )

    out = np.empty((B, C, N), dtype=np.float32)
    for core in range(NCORES):
        b, h = divmod(core, 2)
        out[b, :, h * NH:(h + 1) * NH] = res.results[core]["out"]
    return out.reshape(B, C, 64, 64)
